# revision 16
# baseline (speedup 1.0000x reference)
"""Trainium2 Bass kernel for the ExponentialEnvelopes module (v2).

Math (per spin):
    feats[n,k]  = [charge, centered coords]           (nuclei features, [128, 4])
    Z[n,o]      = (feats @ W_pi)[n,o]                 (= zeta.T)
    P[n,o]      = (feats @ W_zeta)[n,o]               (= pi.T)
    d[e,n]      = ||e_coords[e] - nuc_coords[n]||
    orb[e,o]    = sum_n P[n,o] * exp(-d[e,n] * |Z[n,o]|)

v2 changes vs v1 (147us): the scalar-engine exp stream (119us busy) is
split three ways per electron over the 4096 orbital columns:
  - cols [0:NB)    "fast" region on DVE: Schraudolph exp2-bitcast
        u_f16 = absz16*( -1477.3*d ) + 15300   (tensor_scalar mult+add)
        t_u16 = convert(u_f16)                 (tensor_copy, 4x mode;
                                                negative -> saturates to 0)
        bitcast u16 -> f16 IS the approx of exp(-d|z|) (rel err ~3%)
  - cols [NB:PB)   ACT exp (f32 absz input, per-partition scale -d)
  - pi-multiply: DVE TT over [0:PB), GPSIMD TT over [PB:4096)
The ~3% fast-path error is made harmless by a host-side column
permutation per (core, spin): columns are ranked by the amplitude bound
M[o] = sum_n |pi| * exp(-dmin_n * |z|) and the smallest-amplitude NB
columns are routed to the fast region (their absolute error is tiny
relative to the output absmax ~160).

Sharding: electrons sharded across 8 cores (16/core, both spins),
orbitals whole per core.  Host gathers per-core [2, 16, 4096] slabs and
un-permutes columns.
"""

import numpy as np
from contextlib import ExitStack

NE = 128          # electrons per spin (total)
NN = 128          # nuclei
NDET = 32
NORB = 4096       # n_det * max_e
N_CORES = 8
E_PER_CORE = NE // N_CORES   # 16
WBLK = NORB // 512           # zeta/pi matmul blocks of 512

NB = 512          # fast-path (Schraudolph) columns, at the END of the 4096
NA = 4096 - NB    # ACT-path columns [0:NA)
A_EXP = 1477.3197  # 1024 / ln(2)
B_EXP = 15360.0 - 60.0  # exponent bias minus Schraudolph mean-centering

_CACHE = {}

LAST_RESULTS = None  # BassKernelResults of the most recent run (for test harness)


def _split_multiwaits(nc, blocks):
    """Every TPB engine instruction has exactly ONE embedded sync-wait slot
    (NEURON_ISA_TPB_EVENTS); Tile's sem assignment can emit several waits on
    one instruction, which walrus rejects ("Too many sync wait commands").
    Hoist all but the last wait onto fresh single-wait NOPs inserted just
    before the instruction on the same engine stream."""
    from concourse import mybir

    for bb, insts in blocks.items():
        out = []
        changed = False
        for inst in insts:
            si = getattr(inst, "sync_info", None)
            waits = list(si.on_wait) if si is not None and si.on_wait else []
            if len(waits) > 1:
                for w in waits[:-1]:
                    nop = mybir.InstNoOp(
                        name=nc.get_next_instruction_name(), ins=[], outs=[])
                    nop.engine = inst.engine
                    nop.sync_info = mybir.SyncInfo(on_wait=[w], on_update=[])
                    out.append(nop)
                inst.sync_info = mybir.SyncInfo(
                    on_wait=[waits[-1]], on_update=list(si.on_update))
                changed = True
            out.append(inst)
        if changed:
            insts[:] = out


def _build_module():
    import concourse.bass as bass
    import concourse.tile as tile
    from concourse import mybir
    from concourse.alu_op_type import AluOpType

    class FixupTileContext(tile.TileContext):
        def _lower_ordered_insts(self, postordered_blocks):
            _split_multiwaits(self.nc, postordered_blocks)
            return super()._lower_ordered_insts(postordered_blocks)

        def _drain_and_barrier(self, tick_clock, wait_clock):
            # The kernel-tail drain waits on the full global clock (~11 sems),
            # over the single embedded wait slot.  Pre-observe the clock on
            # the sync engine via single-wait NOPs; add_sem_waits then elides
            # the (now redundant) waits on the real drain.
            from concourse.vector_clock import ScopedClock

            probe = self.nc.sync.nop()
            wait_clock.add_sem_waits(
                probe.ins, ScopedClock({None: tick_clock.global_clock}))
            si = probe.ins.sync_info
            waits = list(si.on_wait) if si is not None and si.on_wait else []
            if len(waits) > 1:
                probe.ins.sync_info = mybir.SyncInfo(
                    on_wait=[waits[0]], on_update=list(si.on_update or []))
                for w in waits[1:]:
                    extra = self.nc.sync.nop()
                    extra.ins.sync_info = mybir.SyncInfo(
                        on_wait=[w], on_update=[])
            ret = super()._drain_and_barrier(tick_clock, wait_clock)
            for blk in self.nc.m.functions[0].blocks:
                for i in blk.instructions:
                    si = getattr(i, "sync_info", None)
                    if (isinstance(i, mybir.InstDrain) and si is not None
                            and si.on_wait and len(si.on_wait) > 1):
                        i.sync_info = mybir.SyncInfo(
                            on_wait=[], on_update=list(si.on_update or []))
            return ret

    f32 = mybir.dt.float32
    f16 = mybir.dt.float16
    u16 = mybir.dt.uint16
    u32 = mybir.dt.uint32
    AF = mybir.ActivationFunctionType
    AX = mybir.AxisListType.X
    E = E_PER_CORE

    nc = bass.Bass(trn_type="TRN2")

    # host-precomputed nuclei features [charge; centered coords] (lhsT for
    # the zeta/pi matmuls) and per-electron distance scales
    d_feats = nc.dram_tensor("feats", [4, NN], f16, kind="ExternalInput")
    # [128 nuclei, 4*E]: [-d spin0 | -A*d spin0 | -d spin1 | -A*d spin1]
    d_negd = nc.dram_tensor("negd", [NN, 4 * E], f32, kind="ExternalInput")
    # W matrices pre-split by the host into charge rows (k=0) and coord rows
    # (k=1..3) so every SBUF access pattern starts at partition 0; all four
    # matrices are packed along the free dim: index (s, m) at (2*s+m)*NORB.
    d_w4 = nc.dram_tensor("w4", [4, 4 * NORB], f16, kind="ExternalInput")
    # per-core output slab: [spin][e_local][orbital] (directly in orb layout)
    d_out = nc.dram_tensor("out", [2, E, NORB], f32, kind="ExternalOutput")

    with ExitStack() as ctx:
        tc = ctx.enter_context(FixupTileContext(nc))
        const = ctx.enter_context(tc.tile_pool(name="const", bufs=1))
        wpool = ctx.enter_context(tc.tile_pool(name="wload", bufs=1))
        tpool = ctx.enter_context(tc.tile_pool(name="texp", bufs=7))
        upool = ctx.enter_context(tc.tile_pool(name="utmp", bufs=4))
        opool = ctx.enter_context(tc.tile_pool(name="outsb", bufs=8))
        psum = ctx.enter_context(tc.tile_pool(name="ps", bufs=1, space="PSUM"))
        _bk = [0]

        def ps_tile(shape, tag=None):
            if tag is None:
                tag = f"bk{_bk[0] % 8}"
            _bk[0] += 1
            return psum.tile(shape, f32, tag=tag, name=f"ps{_bk[0]}_{tag}")

        # ---------------- small loads ----------------
        s_f16 = const.tile([4, NN], f16, tag="feats16")
        nc.sync.dma_start(s_f16[:], d_feats[:])
        s_nd = const.tile([NN, 4 * E], f32, tag="negd")
        nc.sync.dma_start(s_nd[:], d_negd[:])
        # W quarter 0 first (spin0-zeta gates the exp stream)
        s_w4 = wpool.tile([4, 4 * NORB], f16, tag="w4")
        for q in range(4):
            qs = slice(q * NORB, (q + 1) * NORB)
            nc.sync.dma_start(s_w4[:, qs], d_w4[:, qs])

        # ---------------- zeta / pi ----------------
        # absz16: fast-region |zeta| in f16   [128, NB]
        # absz32: ACT-region |zeta| in f32    [128, NORB-NB]
        # piT:    pi in f16                   [128, NORB]
        s_absz16 = []
        s_absz32 = []
        s_piT = []
        for s in (0, 1):
            s_absz16.append(const.tile([128, NB], f16, tag=f"a16{s}",
                                       name=f"a16{s}"))
            s_absz32.append(const.tile([128, NA], f32, tag=f"a32{s}",
                                       name=f"a32{s}"))
            s_piT.append(const.tile([128, NORB], f16, tag=f"pit{s}",
                                    name=f"pit{s}"))
        # Pool's private copy of the fast-region pi slice: keeps GPSIMD off
        # the piT tile's SBUF ports while DVE streams the big quarters
        s_piTf = [const.tile([128, NB], f16, tag=f"pitf{s}", name=f"pitf{s}")
                  for s in (0, 1)]

        def w_matmul(dst_ps, w_off, blk):
            sl = slice(w_off + blk * 512, w_off + (blk + 1) * 512)
            nc.tensor.matmul(dst_ps[:], lhsT=s_f16[:], rhs=s_w4[:, sl],
                             start=True, stop=True)

        def emit_zeta_blk(s, blk):
            """Postprocess zeta block -> absz32 (blocks 0-6) / absz16 (blk 7)."""
            lo, hi = blk * 512, (blk + 1) * 512
            ps_z = ps_tile([128, 512])
            w_matmul(ps_z, (2 * s) * NORB, blk)       # zeta uses W_pi
            if lo >= NA:
                # fast region: f16 |zeta| via ACT Abs (spin1: DVE to keep ACT
                # free during spin0's exp stream)
                if s == 0:
                    nc.scalar.activation(s_absz16[s][:], ps_z[:], AF.Abs)
                else:
                    s_tmp16 = const.tile([128, NB], f16, tag=f"ztmp{s}")
                    nc.vector.tensor_copy(s_tmp16[:], ps_z[:])
                    nc.vector.tensor_scalar(
                        s_absz16[s][:].bitcast(u16), s_tmp16[:].bitcast(u16),
                        0x7FFF, None, AluOpType.bitwise_and)
            else:
                # ACT-region |zeta| in f32; alternate engines in the head
                # (spin1 stays off ACT: its abs runs during spin0's stream)
                if s == 1 or blk % 2 == 0:
                    nc.vector.tensor_scalar(
                        s_absz32[s][:, lo:hi].bitcast(u32),
                        ps_z[:].bitcast(u32),
                        0x7FFFFFFF, None, AluOpType.bitwise_and)
                else:
                    nc.scalar.activation(s_absz32[s][:, lo:hi],
                                         ps_z[:], AF.Abs)

        def emit_pi_blk(s, blk):
            sl = slice(blk * 512, (blk + 1) * 512)
            ps_p = ps_tile([128, 512])
            w_matmul(ps_p, (2 * s + 1) * NORB, blk)   # pi uses W_zeta
            nc.vector.tensor_copy(s_piT[s][:, sl], ps_p[:])
            if blk == WBLK - 1:
                nc.vector.tensor_copy(s_piTf[s][:], ps_p[:, 512 - NB:])

        # spin0's zeta first (needed to start its exp stream), then the rest.
        # The very first exp's halves are emitted INSIDE the zeta loop so the
        # strict ACT FIFO doesn't queue them behind later abs ops.
        t_act0 = tpool.tile([128, NA], f16, tag="T", name="tact_e0")
        Hh = 2048
        e0_pieces = {1: (0, 1024), 3: (1024, 2048), 5: (2048, 3072),
                     6: (3072, NA)}
        for blk in range(WBLK):
            emit_zeta_blk(0, blk)
            if blk in e0_pieces:
                lo, hi = e0_pieces[blk]
                nc.scalar.activation(t_act0[:, lo:hi],
                                     s_absz32[0][:, lo:hi],
                                     AF.Exp, scale=s_nd[:, 0:1])
        for blk in range(WBLK):
            emit_pi_blk(0, blk)
        for blk in range(WBLK):
            emit_zeta_blk(1, blk)
        for blk in range(WBLK):
            emit_pi_blk(1, blk)

        # One-hot selector: lhsT slice e is [128, E] with column e all-ones.
        s_oh = const.tile([128, E * E], f16, tag="onehot")
        nc.vector.memset(s_oh[:], 0.0)
        for e in range(E):
            nc.vector.memset(s_oh[:, e * E + e:e * E + e + 1], 1.0)

        # ---------------- main loop ----------------
        NCHUNK = NORB // 512   # 8 psum banks, one per 512-orbital chunk

        def emit_fast(s, e, last):
            """Schraudolph fast path for cols [NA:4096): returns t_fast.
            Emitted one electron AHEAD of its matmul so the Pool TT is never
            on the PE critical path."""
            s_v = upool.tile([128, NB], f16, tag="V")
            nc.vector.tensor_scalar(s_v[:], s_absz16[s][:],
                                    s_nd[:, 2 * E * s + E + e:2 * E * s + E + e + 1], None,
                                    AluOpType.mult)
            s_u = upool.tile([128, NB], u16, tag="U")
            nc.vector.tensor_scalar(s_u[:], s_v[:], -B_EXP, B_EXP,
                                    AluOpType.max, AluOpType.add)
            t_fast = upool.tile([128, NB], f16, tag="F")
            nc.vector.tensor_mul(t_fast[:], s_u[:].bitcast(f16), s_piTf[s][:])
            return t_fast

        for s in (0, 1):
            ps_orb = [ps_tile([E, 512], tag=f"bk{c}") for c in range(NCHUNK)]
            t_fast_next = emit_fast(s, 0, False)
            for e in range(E):
                last = (e == E - 1)
                t_fast = t_fast_next
                if s == 0 and e == 0:
                    t_act = t_act0   # ACT-region exp already emitted (hoisted)
                else:
                    t_act = tpool.tile([128, NA], f16, tag="T")
                    if last:
                        # quarter the final exp: evac chain starts ~2.4us
                        # earlier off the last chunk matmuls
                        for lo, hi in ((0, 1024), (1024, 2048),
                                       (2048, 3072), (3072, NA)):
                            nc.scalar.activation(t_act[:, lo:hi],
                                                 s_absz32[s][:, lo:hi],
                                                 AF.Exp,
                                                 scale=s_nd[:, 2 * E * s + e:2 * E * s + e + 1])
                    elif s == 1 and e == 0:
                        # split boundary exps so downstream work starts earlier
                        nc.scalar.activation(t_act[:, 0:Hh],
                                             s_absz32[s][:, 0:Hh],
                                             AF.Exp, scale=s_nd[:, 2 * E * s + e:2 * E * s + e + 1])
                        nc.scalar.activation(t_act[:, Hh:NA],
                                             s_absz32[s][:, Hh:NA],
                                             AF.Exp, scale=s_nd[:, 2 * E * s + e:2 * E * s + e + 1])
                    else:
                        nc.scalar.activation(t_act[:, 0:NA],
                                             s_absz32[s][:, 0:],
                                             AF.Exp, scale=s_nd[:, 2 * E * s + e:2 * E * s + e + 1])
                # next electron's fast path, queued on DVE before this
                # electron's quarters so Pool gets its input early
                if not last:
                    t_fast_next = emit_fast(s, e + 1, e + 1 == E - 1)
                # chunk 7's matmul first: its rhs was produced last iteration
                nc.tensor.matmul(ps_orb[7][:],
                                 lhsT=s_oh[:, e * E:(e + 1) * E],
                                 rhs=t_fast[:],
                                 start=(e == 0), stop=(e == E - 1))
                # ACT-region pi-weighting: DVE quarters, in-place
                nc.vector.tensor_mul(t_act[:, 0:1024],
                                     t_act[:, 0:1024], s_piT[s][:, 0:1024])
                nc.vector.tensor_mul(t_act[:, 1024:2048],
                                     t_act[:, 1024:2048],
                                     s_piT[s][:, 1024:2048])
                nc.vector.tensor_mul(t_act[:, 2048:3072],
                                     t_act[:, 2048:3072],
                                     s_piT[s][:, 2048:3072])
                nc.vector.tensor_mul(t_act[:, 3072:NA],
                                     t_act[:, 3072:NA],
                                     s_piT[s][:, 3072:NA])
                for c in range(NCHUNK - 1):
                    nc.tensor.matmul(ps_orb[c][:],
                                     lhsT=s_oh[:, e * E:(e + 1) * E],
                                     rhs=t_act[:, c * 512:(c + 1) * 512],
                                     start=(e == 0), stop=(e == E - 1))
            for c in range(NCHUNK):
                s_o = opool.tile([E, 512], f32, tag="osb")
                if c % 2 == 0:
                    nc.vector.tensor_copy(s_o[:], ps_orb[c][:])
                else:
                    nc.scalar.copy(s_o[:], ps_orb[c][:])
                # spread the final evac DMAs across two queues
                dma_eng = nc.scalar if (s == 1 and c % 2 == 1) else nc.sync
                dma_eng.dma_start(d_out[s][:, c * 512:(c + 1) * 512], s_o[:])

    return nc


def _get_module():
    if "nc" not in _CACHE:
        _CACHE["nc"] = _build_module()
    return _CACHE["nc"]


def _column_orders(up, down, nuc, chg, w):
    """Per (core, spin) orbital-column permutation: ascending amplitude
    bound M[o] = sum_n |pi[n,o]| exp(-dmin_n |z[n,o]|), dmin over the
    core's electrons.  Low-amplitude columns go first -> fast region."""
    cen = nuc - nuc.mean(0, keepdims=True)
    feats = np.concatenate([chg[:, None], cen], -1)        # [128, 4]
    orders = np.empty((N_CORES, 2, NORB), dtype=np.int64)
    for s, (ec, wp, wz) in enumerate(
            [(up, w["W_pi_up"], w["W_zeta_up"]),
             (down, w["W_pi_down"], w["W_zeta_down"])]):
        z = feats @ wp                                     # [128, NORB]
        pi = feats @ wz
        absz = np.abs(z)
        api = np.abs(pi)
        for c in range(N_CORES):
            el = ec[c * E_PER_CORE:(c + 1) * E_PER_CORE]   # [E, 3]
            dist = np.sqrt(((el[:, None, :] - nuc[None, :, :]) ** 2).sum(-1))
            dmin = dist.min(0)                             # [128]
            M = (api * np.exp(-dmin[:, None] * absz)).sum(0)
            orders[c, s] = np.argsort(-M, kind="stable")
    return orders


def kernel(**inputs) -> np.ndarray:
    global LAST_RESULTS
    nc = _get_module()
    from concourse.bass_utils import run_bass_kernel_spmd

    up = np.ascontiguousarray(np.asarray(inputs["up_coords"], dtype=np.float32))
    down = np.ascontiguousarray(np.asarray(inputs["down_coords"], dtype=np.float32))
    nuc = np.asarray(inputs["nuc_coords"], dtype=np.float32)
    chg = np.asarray(inputs["nuc_charges"], dtype=np.float32)
    w = {
        k: np.ascontiguousarray(np.asarray(inputs[k], dtype=np.float32))
        for k in ("W_pi_up", "W_zeta_up", "W_pi_down", "W_zeta_down")
    }
    orders = _column_orders(up, down, nuc, chg, w)

    cen = nuc - nuc.mean(0, keepdims=True)
    feats16 = np.ascontiguousarray(
        np.concatenate([chg[None, :], cen.T], 0).astype(np.float16))  # [4,128]

    in_maps = []
    for c in range(N_CORES):
        sl = slice(c * E_PER_CORE, (c + 1) * E_PER_CORE)
        negd = np.empty((NN, 4 * E_PER_CORE), dtype=np.float32)
        for s, ec in enumerate((up, down)):
            dist = np.sqrt(((ec[sl][None, :, :] - nuc[:, None, :]) ** 2)
                           .sum(-1))                       # [128 nuc, E]
            negd[:, 2 * E_PER_CORE * s:2 * E_PER_CORE * s + E_PER_CORE] = -dist
            negd[:, 2 * E_PER_CORE * s + E_PER_CORE:
                 2 * E_PER_CORE * (s + 1)] = -A_EXP * dist
        ou, od = orders[c, 0], orders[c, 1]
        w4 = np.concatenate(
            [w["W_pi_up"][:, ou], w["W_zeta_up"][:, ou],
             w["W_pi_down"][:, od], w["W_zeta_down"][:, od]],
            axis=1).astype(np.float16)
        in_maps.append({"feats": feats16, "negd": negd,
                        "w4": np.ascontiguousarray(w4)})

    res = run_bass_kernel_spmd(nc, in_maps, core_ids=list(range(N_CORES)))
    LAST_RESULTS = res

    # gather: per-core slab is [2, e_local, orbital(permuted)]
    orb = np.empty((2, NE, NORB), dtype=np.float32)
    for c in range(N_CORES):
        a = np.asarray(res.results[c]["out"])            # [2, E, NORB]
        esl = slice(c * E_PER_CORE, (c + 1) * E_PER_CORE)
        orb[0, esl, orders[c, 0]] = a[0].T
        orb[1, esl, orders[c, 1]] = a[1].T

    # [2, n_e, n_det*max_e] -> [2, n_det, n_e, max_e]
    out = orb.reshape(2, NE, NDET, NE).swapaxes(1, 2)
    return np.ascontiguousarray(out)
